# revision 1
# baseline (speedup 1.0000x reference)
"""CorrNoise kernel for 8x TRN2 NeuronCores.

Reference computation: center/normalize ref over batch -> per-dim (l x l)
correlation -> eigh -> out[d] = (Q*sqrt(max(eig,0)))[d] @ noise[d].

Split of work:
  * corr + eigh run on HOST with jax on CPU, mirroring the reference ops
    bit-exactly.  This is forced: (a) eigh has no neuron lowering at all;
    (b) LAPACK eigenvector SIGNS are implementation-defined and flip under
    ~1e-7 input perturbations, and the output is sign-sensitive, so the
    eigh input must be bit-identical to the reference's and the eigh must
    be the same LAPACK build (jnp.linalg.eigh on CPU).
  * The post-eigh work - 512 independent (128x128)@(128x256) GEMMs - runs
    on the 8 NeuronCores, sharded by dim (64 per core).

Device kernel design (measured on HW via NTFF profiles):
  * fp16x3 decomposition: each fp32 operand is split (on host, free) into
    hi+lo fp16 planes; out = Wh@Xh + Wh@Xl + Wl@Xh accumulated in fp32
    PSUM.  Same DMA bytes as fp32 (2x fp16 = 4B), but 16-bit matmuls
    stream at 1 cycle/row vs fp32's 2 half-rate passes (4 cyc/row).
    End-to-end rel err vs the fp32 reference matmul: 2.4e-07.
  * All input tiles and output tiles are SBUF-resident (no buffer reuse),
    so there are no backward scheduling edges: input DMAs (sync ring)
    never wait on compute, output DMAs (alternating rings) never block
    inputs.  DMA streams back-to-back at ~425 GB/s (fabric ceiling),
    which is the binding resource: 21 MB/core total traffic.
  * hi and lo planes are fused into ONE SBUF tile per 8-dim group, so
    each group loads with a single 1.57 MB DMA (8 input DMAs total -
    fewer issues and HWDGE-lane serializations).  The first group's load
    is split in half for an earlier PE start; the last output DMA is
    split in half to shorten the tail.
"""

import numpy as np

EPS = 1e-5
SIZE = 128   # l: corr matrices are SIZE x SIZE
DIM = 512    # d: number of independent feature dims
BATCH = 256  # b
NCORES = 8
DPC = DIM // NCORES  # dims per core
GRP = 8              # dims per load/store group
NGRP = DPC // GRP
WX = SIZE + BATCH    # packed per-dim columns: [QS^T | noise]

_cache = {}


def _host_qs(ref: np.ndarray) -> np.ndarray:
    """Bit-exact mirror of the reference's pre-matmul stages on jax CPU.

    Returns QS = Ds[:, None, :] * Qs with shape (DIM, SIZE, SIZE), fp32.
    """
    import jax
    import jax.numpy as jnp

    cpu = jax.devices("cpu")[0]
    with jax.default_device(cpu):
        refj = jnp.asarray(np.asarray(ref, dtype=np.float32))
        x = refj - refj.mean(axis=0, keepdims=True)
        x = x / (jnp.linalg.norm(x, axis=0, keepdims=True) + EPS)
        x = jnp.transpose(x, (2, 1, 0))  # (d, l, b)
        corr = jnp.einsum("dlb,dmb->dlm", x, x)  # (d, l, l)
        i = jnp.arange(SIZE)
        corr = corr.at[:, i, i].set(1.0)
        Ds, Qs = jnp.linalg.eigh(corr)  # Ds: (d, l), Qs: (d, l, l)
        Ds = jnp.sqrt(jnp.maximum(Ds, 0.0))
        Qs = Ds[:, None, :] * Qs
        return np.asarray(Qs)


def _build_nc():
    import concourse.bass as bass
    import concourse.tile as tile
    from concourse import bacc, mybir

    f32 = mybir.dt.float32
    f16 = mybir.dt.float16
    W2 = 2 * GRP * WX  # fused row: [hi plane | lo plane]
    nc = bacc.Bacc("TRN2", target_bir_lowering=False, debug=False,
                   num_devices=NCORES)
    # wx2[g, p, j*WX + c]         : fp16 hi plane of [QS[d].T | noise_t[d]]
    # wx2[g, p, LO + j*WX + c]    : fp16 lo plane, d = g*GRP + j
    # (host interleaves so every DMA is a contiguous 2D copy)
    wx2 = nc.dram_tensor("wx2", [NGRP, SIZE, W2], f16,
                         kind="ExternalInput").ap()
    out = nc.dram_tensor("out", [NGRP, SIZE, GRP * BATCH], f32,
                         kind="ExternalOutput").ap()
    with tile.TileContext(nc) as tc:
        with (
            tc.tile_pool(name="wx2", bufs=NGRP) as wxp,
            tc.tile_pool(name="o", bufs=NGRP) as op_,
            tc.tile_pool(name="ps", bufs=7, space=bass.MemorySpace.PSUM) as pp,
        ):
            ts = []
            for g in range(NGRP):
                t = wxp.tile([SIZE, W2], f16)
                ns = 2 if g == 0 else 1  # split first load: earlier PE start
                step = W2 // ns
                for s in range(ns):
                    nc.sync.dma_start(t[:, s * step:(s + 1) * step],
                                      wx2[g, :, s * step:(s + 1) * step])
                ts.append(t)
            LO = GRP * WX
            for g in range(NGRP):
                t = ts[g]
                o = op_.tile([SIZE, GRP * BATCH], f32)
                for j in range(GRP):
                    wh = t[:, j * WX:j * WX + SIZE]
                    xh = t[:, j * WX + SIZE:(j + 1) * WX]
                    wl = t[:, LO + j * WX:LO + j * WX + SIZE]
                    xl = t[:, LO + j * WX + SIZE:LO + (j + 1) * WX]
                    ps = pp.tile([SIZE, BATCH], f32)
                    nc.tensor.matmul(ps[:], wh, xh, start=True, stop=False)
                    nc.tensor.matmul(ps[:], wh, xl, start=False, stop=False)
                    nc.tensor.matmul(ps[:], wl, xh, start=False, stop=True)
                    nc.vector.tensor_copy(o[:, j * BATCH:(j + 1) * BATCH],
                                          ps[:])
                eng = nc.scalar if g % 2 == 0 else nc.sync
                if g < NGRP - 1:
                    eng.dma_start(out[g], o[:])
                else:  # split the last store: shorter tail
                    half = GRP * BATCH // 2
                    nc.scalar.dma_start(out[g, :, :half], o[:, :half])
                    nc.sync.dma_start(out[g, :, half:], o[:, half:])
    nc.compile()
    return nc


def _run_device(qst: np.ndarray, noise_t: np.ndarray, trace: bool = False):
    """qst: (DIM, SIZE, SIZE) = QS transposed per dim (fp32);
    noise_t: (DIM, SIZE, BATCH) fp32.
    Returns (out_t (DIM, SIZE, BATCH) fp32, BassKernelResults)."""
    from concourse.bass_utils import run_bass_kernel_spmd

    if "nc" not in _cache:
        _cache["nc"] = _build_nc()
    nc = _cache["nc"]

    wx = np.concatenate([qst, noise_t], axis=2)  # (DIM, SIZE, WX) f32
    wx = wx.reshape(NCORES, NGRP, GRP, SIZE, WX).transpose(0, 1, 3, 2, 4)
    wx = np.ascontiguousarray(wx).reshape(NCORES, NGRP, SIZE, GRP * WX)
    wxh = wx.astype(np.float16)
    wxl = (wx - wxh.astype(np.float32)).astype(np.float16)
    wx2 = np.concatenate([wxh, wxl], axis=3)  # (NC, NGRP, SIZE, 2*GRP*WX)
    in_maps = [{"wx2": np.ascontiguousarray(wx2[c])} for c in range(NCORES)]
    res = run_bass_kernel_spmd(nc, in_maps, list(range(NCORES)), trace=trace)
    out_t = np.stack([res.results[c]["out"] for c in range(NCORES)])
    out_t = out_t.reshape(NCORES, NGRP, SIZE, GRP, BATCH)
    out_t = out_t.transpose(0, 1, 3, 2, 4).reshape(DIM, SIZE, BATCH)
    return out_t, res


def kernel(standard_noise: np.ndarray, ref: np.ndarray) -> np.ndarray:
    qs = _host_qs(ref)  # (d, l, l)
    qst = np.ascontiguousarray(np.transpose(qs, (0, 2, 1)))
    noise_t = np.ascontiguousarray(
        np.transpose(np.asarray(standard_noise, dtype=np.float32), (2, 1, 0)))
    out_t, _ = _run_device(qst, noise_t)
    return np.ascontiguousarray(np.transpose(out_t, (2, 1, 0)))



# revision 2
# speedup vs baseline: 1.6033x; 1.6033x over previous
"""CorrNoise kernel for 8x TRN2 NeuronCores.

Reference computation: center/normalize ref over batch -> per-dim (l x l)
correlation -> eigh -> out[d] = (Q*sqrt(max(eig,0)))[d] @ noise[d].

Split of work:
  * corr + eigh run on HOST with jax on CPU, mirroring the reference ops
    bit-exactly.  This is forced: (a) eigh has no neuron lowering at all;
    (b) LAPACK eigenvector SIGNS are implementation-defined and flip under
    ~1e-7 input perturbations, and the output is sign-sensitive, so the
    eigh input must be bit-identical to the reference's and the eigh must
    be the same LAPACK build (jnp.linalg.eigh on CPU).
  * The post-eigh work - 512 independent (128x128)@(128x256) GEMMs - runs
    on the 8 NeuronCores, sharded by dim (64 per core).

Device kernel design (measured on HW via NTFF profiles):
  * Single-plane fp16: operands are rounded to fp16 on host, one matmul
    per dim accumulating in fp32 PSUM, output stored as fp16 and upcast
    on host.  End-to-end rel err vs the fp32 reference: ~3.6e-4, far
    inside the 2e-2 gate, and it halves the DMA traffic vs the old
    fp16x3 hi/lo scheme (10.5 MB/core vs 21 MB/core).  DMA is the
    binding resource (per-core HBM ~358 GB/s), so bytes ~= time.
  * All input tiles and output tiles are SBUF-resident (no buffer reuse),
    so there are no backward scheduling edges: input DMAs (sync ring)
    never wait on compute, output DMAs (alternating rings) never block
    inputs.
  * Each 8-dim group loads with a single 786 KB DMA (8 input DMAs total).
    The first group's load is split in half for an earlier PE start; the
    last output DMA is split in half to shorten the tail.
  * PSUM->SBUF drains (with the fp32->fp16 cast) alternate between the
    vector and scalar engines so the drain never gates the output DMAs.
"""

import numpy as np

EPS = 1e-5
SIZE = 128   # l: corr matrices are SIZE x SIZE
DIM = 512    # d: number of independent feature dims
BATCH = 256  # b
NCORES = 8
DPC = DIM // NCORES  # dims per core
GRP = 8              # dims per load/store group
NGRP = DPC // GRP
WX = SIZE + BATCH    # packed per-dim columns: [QS^T | noise]

_cache = {}


def _host_qs(ref: np.ndarray) -> np.ndarray:
    """Bit-exact mirror of the reference's pre-matmul stages on jax CPU.

    Returns QS = Ds[:, None, :] * Qs with shape (DIM, SIZE, SIZE), fp32.
    """
    import jax
    import jax.numpy as jnp

    cpu = jax.devices("cpu")[0]
    with jax.default_device(cpu):
        refj = jnp.asarray(np.asarray(ref, dtype=np.float32))
        x = refj - refj.mean(axis=0, keepdims=True)
        x = x / (jnp.linalg.norm(x, axis=0, keepdims=True) + EPS)
        x = jnp.transpose(x, (2, 1, 0))  # (d, l, b)
        corr = jnp.einsum("dlb,dmb->dlm", x, x)  # (d, l, l)
        i = jnp.arange(SIZE)
        corr = corr.at[:, i, i].set(1.0)
        Ds, Qs = jnp.linalg.eigh(corr)  # Ds: (d, l), Qs: (d, l, l)
        Ds = jnp.sqrt(jnp.maximum(Ds, 0.0))
        Qs = Ds[:, None, :] * Qs
        return np.asarray(Qs)


def _build_nc():
    import concourse.bass as bass
    import concourse.tile as tile
    from concourse import bacc, mybir

    f16 = mybir.dt.float16
    f32 = mybir.dt.float32
    W = GRP * WX  # packed row: GRP dims of [QS^T | noise]
    nc = bacc.Bacc("TRN2", target_bir_lowering=False, debug=False,
                   num_devices=NCORES)
    # wx[g, p, j*WX + c] : fp16 plane of [QS[d].T | noise_t[d]], d = g*GRP+j
    wx = nc.dram_tensor("wx", [NGRP, SIZE, W], f16,
                        kind="ExternalInput").ap()
    out = nc.dram_tensor("out", [NGRP, SIZE, GRP * BATCH], f16,
                         kind="ExternalOutput").ap()
    with tile.TileContext(nc) as tc:
        with (
            tc.tile_pool(name="wx", bufs=NGRP) as wxp,
            tc.tile_pool(name="o", bufs=NGRP) as op_,
            tc.tile_pool(name="ps", bufs=8, space=bass.MemorySpace.PSUM) as pp,
        ):
            ts = []
            for g in range(NGRP):
                t = wxp.tile([SIZE, W], f16)
                ns = 2 if g == 0 else 1  # split first load: earlier PE start
                step = W // ns
                for s in range(ns):
                    nc.sync.dma_start(t[:, s * step:(s + 1) * step],
                                      wx[g, :, s * step:(s + 1) * step])
                ts.append(t)
            for g in range(NGRP):
                t = ts[g]
                o = op_.tile([SIZE, GRP * BATCH], f16)
                for j in range(GRP):
                    wh = t[:, j * WX:j * WX + SIZE]
                    xh = t[:, j * WX + SIZE:(j + 1) * WX]
                    ps = pp.tile([SIZE, BATCH], f32)
                    nc.tensor.matmul(ps[:], wh, xh, start=True, stop=True)
                    dst = o[:, j * BATCH:(j + 1) * BATCH]
                    if j % 2 == 0:
                        nc.vector.tensor_copy(dst, ps[:])
                    else:
                        nc.scalar.copy(dst, ps[:])
                eng = nc.scalar if g % 2 == 0 else nc.sync
                if g < NGRP - 1:
                    eng.dma_start(out[g], o[:])
                else:  # split the last store: shorter tail
                    half = GRP * BATCH // 2
                    nc.scalar.dma_start(out[g, :, :half], o[:, :half])
                    nc.sync.dma_start(out[g, :, half:], o[:, half:])
    nc.compile()
    return nc


def _run_device(qst: np.ndarray, noise_t: np.ndarray, trace: bool = False):
    """qst: (DIM, SIZE, SIZE) = QS transposed per dim (fp32);
    noise_t: (DIM, SIZE, BATCH) fp32.
    Returns (out_t (DIM, SIZE, BATCH) fp32, BassKernelResults)."""
    from concourse.bass_utils import run_bass_kernel_spmd

    if "nc" not in _cache:
        _cache["nc"] = _build_nc()
    nc = _cache["nc"]

    wx = np.concatenate([qst, noise_t], axis=2)  # (DIM, SIZE, WX) f32
    wx = wx.reshape(NCORES, NGRP, GRP, SIZE, WX).transpose(0, 1, 3, 2, 4)
    wx = np.ascontiguousarray(wx).reshape(NCORES, NGRP, SIZE, GRP * WX)
    wxh = wx.astype(np.float16)
    in_maps = [{"wx": np.ascontiguousarray(wxh[c])} for c in range(NCORES)]
    res = run_bass_kernel_spmd(nc, in_maps, list(range(NCORES)), trace=trace)
    out_t = np.stack([res.results[c]["out"] for c in range(NCORES)])
    out_t = out_t.reshape(NCORES, NGRP, SIZE, GRP, BATCH)
    out_t = out_t.transpose(0, 1, 3, 2, 4).reshape(DIM, SIZE, BATCH)
    return out_t.astype(np.float32), res


def kernel(standard_noise: np.ndarray, ref: np.ndarray) -> np.ndarray:
    qs = _host_qs(ref)  # (d, l, l)
    qst = np.ascontiguousarray(np.transpose(qs, (0, 2, 1)))
    noise_t = np.ascontiguousarray(
        np.transpose(np.asarray(standard_noise, dtype=np.float32), (2, 1, 0)))
    out_t, _ = _run_device(qst, noise_t)
    return np.ascontiguousarray(np.transpose(out_t, (2, 1, 0)))
